# revision 72
# baseline (speedup 1.0000x reference)
"""Trainium2 Bass kernel for nn_MultiHeadAttention_73607149519012.

MHA: B=8, S=1024, D=1024, H=16 heads, depth=64, fp32 in/out.
Sharding: data-parallel over batch -- one batch element per NeuronCore (8 cores).

fp8e4 DoubleRow matmuls everywhere except the dominant q_in @ Wo_top half
(bf16).  DoubleRow packs two K-tiles per instruction at 0.5 cycles/row, so the
fp8 matmuls run ~4x faster than bf16 per MAC.  The attention path contributes
only ~2% of the output magnitude (ctx std ~0.02 vs q_in std ~1), so fp8 noise
there is diluted ~50x and the dominant half stays bf16.

Layouts (all host-prepared, no on-device transposes):
  Q/K proj : psum[dout',s] = Wq'^T x^T  with Wq' column-permuted so each psum
             chunk drains 1:1 into the DoubleRow logits layout
             qt[tile][32*(h%4)+d%32, d//32, s]  (4 heads per 128-partition tile)
  V proj   : psum[s,dv] -> v65p[kt//2][s, kt%2, head, 0:64]; column 64 holds
             mask[kpos]/32 so the PV matmul accumulates rowsum/32 in row 64
  logits   : DR lhsT=KT[32,2,128], rhs=QT[32,2,512] at tile_position (32*(h%4),0)
             -> psum[kpos128, q512];  exp has NO mask term (mask rides V rows)
  exp      : split ACT (exact Exp) / DVE (Schraudolph int8(a*x+b) bitcast fp8,
             rms ~3%, mean ~0; GPSIMD cannot read PSUM so Pool gets none)
  PV       : DR lhsT=v65p[128,2,65], rhs=pts[128,2,512] -> psum[65, 2, q512]
             (both q-chunks in one 2-bank tile); row 64 = rowsum/32;
             reciprocal -> Pool broadcast -> DVE mul gives ctx*32 in fp8
  out      : psum[s,dout] = x@Wo_top (bf16, + K=1 ones-row matmul adding bo)
             drained by ACT to bf16 oa; then psum2 = (32ctx)@(64Wo_bot) fp8 DR
             and final out = psum2/2048 + oa  (scalar_tensor_tensor)

Emission interleaves phases so the exp stream (the ACT/DVE bottleneck) starts
as early as possible: V proj first, then per-tile [Q chunks, K chunks, 4 heads].
"""

import os
from contextlib import ExitStack

import numpy as np
import ml_dtypes

import concourse.mybir as mybir
import concourse.tile as tile
from concourse import bacc
from concourse.bass_utils import run_bass_kernel_spmd

F32 = mybir.dt.float32
BF16 = mybir.dt.bfloat16
F8 = mybir.dt.float8e4
I8 = mybir.dt.int8
AF = mybir.ActivationFunctionType
ALU = mybir.AluOpType
DR = mybir.MatmulPerfMode.DoubleRow

NP8 = ml_dtypes.float8_e4m3
NPBF = ml_dtypes.bfloat16

D = 1024
S = 1024
H = 16
DEP = 64
B = 8
SCALE = 1.0 / 8.0          # 1/sqrt(DEP)
A_SCH = 8.0 / np.log(2.0)  # Schraudolph slope for 3-bit-mantissa fp8
B_SCH = 56.0 - 0.45        # bias 7*8, calibrated -0.45 to zero the mean error
CTX_S = 32.0               # ctx scaled by 32 into fp8 (ones column = 1/32)
WOB_S = 64.0               # Wo_bot scaled by 64 into fp8
OUT_S = 1.0 / (CTX_S * WOB_S)

# exp engine split ACT:DVE:(Pool unused), tunable for balance
_EW = os.environ.get("MHA_EXP_W", "80,48,0")
EXP_W = tuple(int(x) for x in _EW.split(","))

LAST_EXEC_NS = None
LAST_RES = None


def _exp_schedule():
    total = sum(EXP_W)
    counts = [w * 128 // total for w in EXP_W]
    while sum(counts) < 128:
        counts[int(np.argmax([w / (c + 1) for w, c in zip(EXP_W, counts)]))] += 1
    sched = []
    acc = [0.0, 0.0, 0.0]
    for _ in range(128):
        for i in range(3):
            acc[i] += counts[i] / 128.0
        pick = int(np.argmax(acc))
        acc[pick] -= 1.0
        sched.append(pick)
    return sched


def build_nc():
    nc = bacc.Bacc(None, target_bir_lowering=False)

    xq8_d = nc.dram_tensor("xq8", [128, 4, 2, S], F8, kind="ExternalInput")
    xk8_d = nc.dram_tensor("xk8", [128, 4, 2, S], F8, kind="ExternalInput")
    xv8_d = nc.dram_tensor("xv8", [128, 4, 2, S], F8, kind="ExternalInput")
    wq8_d = nc.dram_tensor("wq8", [128, 4, 2, D], F8, kind="ExternalInput")
    wk8_d = nc.dram_tensor("wk8", [128, 4, 2, D], F8, kind="ExternalInput")
    wv8_d = nc.dram_tensor("wv8", [128, 4, 2, D], F8, kind="ExternalInput")
    wob8_d = nc.dram_tensor("wob8", [128, 4, 2, D], F8, kind="ExternalInput")
    xqb_d = nc.dram_tensor("xqb", [128, 8, S], BF16, kind="ExternalInput")
    wot_d = nc.dram_tensor("wot", [128, 8, D], BF16, kind="ExternalInput")
    # cst cols: 0-7 bq' (permuted), 8-15 bk' (permuted), 16-23 mprime per st
    cst_d = nc.dram_tensor("cst", [128, 24], F32, kind="ExternalInput")
    bvr_d = nc.dram_tensor("bvr", [1, D], BF16, kind="ExternalInput")
    bor_d = nc.dram_tensor("bor", [1, D], BF16, kind="ExternalInput")
    idm_d = nc.dram_tensor("idm", [128, 128], BF16, kind="ExternalInput")
    out_d = nc.dram_tensor("out", [S, D], F32, kind="ExternalOutput")

    dbg = os.environ.get("MHA_DBG", "0") == "1"
    if dbg:
        dqt_d = nc.dram_tensor("dqt", [4, 128, 2, S], F8, kind="ExternalOutput")
        dkt_d = nc.dram_tensor("dkt", [4, 128, 2, S], F8, kind="ExternalOutput")
        dv65_d = nc.dram_tensor("dv65", [4, 128, 2, H, 65], F8, kind="ExternalOutput")
        dpts_d = nc.dram_tensor("dpts", [128, 4, 2, S], F8, kind="ExternalOutput")
        dctx_d = nc.dram_tensor("dctx", [4, 128, 2, S], F8, kind="ExternalOutput")
        doa_d = nc.dram_tensor("doa", [8, 128, D], BF16, kind="ExternalOutput")

    exp_sched = _exp_schedule()
    exp_i = [0]

    with tile.TileContext(nc) as tc:
        es = ExitStack()
        cp = es.enter_context(tc.tile_pool(name="cp", bufs=1))
        ap_es = ExitStack()
        ap = ap_es.enter_context(tc.tile_pool(name="ap", bufs=1))

        # ---- persistent tiles ----
        cst = cp.tile([128, 24], F32, name="cst")
        xqb = cp.tile([128, 8, S], BF16, name="xqb")
        wot = cp.tile([128, 8, D], BF16, name="wot")
        bvr = cp.tile([1, D], BF16, name="bvr")
        bor = cp.tile([1, D], BF16, name="bor")
        idm = cp.tile([128, 128], BF16, name="idm")
        ones1 = cp.tile([1, 128], BF16, name="ones1")
        qt = [cp.tile([128, 2, S], F8, name=f"qt{t}", tag=f"qt{t}") for t in range(4)]
        kt = [cp.tile([128, 2, S], F8, name=f"kt{t}", tag=f"kt{t}") for t in range(4)]
        # column 64 holds mask/32 so the PV matmul accumulates the softmax
        # denominator (rowsum/32) in psum row 64
        v65 = [cp.tile([128, 2, H, 65], F8, name=f"v65_{t}", tag=f"v65_{t}") for t in range(4)]
        ctxp = [cp.tile([128, 2, S], F8, name=f"ctx{t}", tag=f"ctx{t}") for t in range(4)]
        oa = [cp.tile([128, D], BF16, name=f"oa{t}", tag=f"oa{t}") for t in range(8)]

        xv8 = ap.tile([128, 4, 2, S], F8, name="xv8")
        wv8 = ap.tile([128, 4, 2, D], F8, name="wv8")
        xq8 = ap.tile([128, 4, 2, S], F8, name="xq8")
        wq8 = ap.tile([128, 4, 2, D], F8, name="wq8")
        xk8 = ap.tile([128, 4, 2, S], F8, name="xk8")
        wk8 = ap.tile([128, 4, 2, D], F8, name="wk8")

        # loads in need-order, big fp8 tensors split per k-tile-pair so the
        # first accumulation steps can start before the full tensor lands
        nc.sync.dma_start(cst, cst_d[:, :])
        nc.sync.dma_start(bvr, bvr_d[:, :])
        nc.gpsimd.memset(ones1, 1.0)
        for t in range(4):
            nc.sync.dma_start(xq8[:, t], xq8_d[:, t])
            nc.sync.dma_start(wq8[:, t], wq8_d[:, t])
        for t in range(4):
            nc.sync.dma_start(xk8[:, t], xk8_d[:, t])
            nc.sync.dma_start(wk8[:, t], wk8_d[:, t])
        for t in range(4):
            nc.sync.dma_start(xv8[:, t], xv8_d[:, t])
            nc.sync.dma_start(wv8[:, t], wv8_d[:, t])
        nc.sync.dma_start(xqb, xqb_d[:, :, :])
        nc.sync.dma_start(wot, wot_d[:, :, :])
        nc.sync.dma_start(bor, bor_d[:, :])
        nc.sync.dma_start(idm, idm_d[:, :])

        # ones/mask column of v65 (column 64 = mprime/32 per kpos), on Pool
        for t in range(4):
            nc.gpsimd.memset(v65[t][:, :, :, 64:65], 1.0 / CTX_S)
        for st in range(8):
            t, b = st // 2, st % 2
            nc.gpsimd.tensor_scalar_mul(
                v65[t][:, b, :, 64:65], v65[t][:, b, :, 64:65],
                cst[:, 16 + st:17 + st])

        psum_es = ExitStack()
        gp = psum_es.enter_context(tc.tile_pool(name="gp", bufs=2, space="PSUM"))

        def emit_outA(gi):
            st, dc = gi // 2, gi % 2
            psA = gp.tile([128, 512], F32, name=f"psA_{st}_{dc}", tag="gp")
            for t8 in range(8):
                nc.tensor.matmul(psA, xqb[:, t8, st * 128:(st + 1) * 128],
                                 wot[:, t8, dc * 512:(dc + 1) * 512],
                                 start=(t8 == 0), stop=False)
            nc.tensor.matmul(psA, ones1, bor[:, dc * 512:(dc + 1) * 512],
                             start=False, stop=True)
            # oa is stored pre-scaled by CTX_S*WOB_S so phase C can add it back
            # into the ph3 psum with a single identity-lhsT matmul
            nc.scalar.activation(oa[st][:, dc * 512:(dc + 1) * 512], psA, AF.Copy,
                                 scale=CTX_S * WOB_S)

        # V projection is emitted inside head 0 (between its exps and its PV)
        # so the exp stream starts as soon as Q/K land; bias rides a K=1
        # ones-row matmul, mask a per-partition Copy/mul scale, drains
        # alternate ACT/DVE to halve the drain-chain latency
        def emit_projV():
            for st in range(8):
                t, b = st // 2, st % 2
                for c in range(2):
                    ps = gp.tile([128, 512], F32, name=f"ps_v_{st}_{c}", tag="gp")
                    for t4 in range(4):
                        nc.tensor.matmul(
                            ps, xv8[:, t4, :, st * 128:(st + 1) * 128],
                            wv8[:, t4, :, c * 512:(c + 1) * 512],
                            start=(t4 == 0), stop=False, perf_mode=DR)
                    nc.tensor.matmul(ps, ones1, bvr[:, c * 512:(c + 1) * 512],
                                     start=False, stop=True)
                    dst = v65[t][:, b, c * 8:(c + 1) * 8, 0:64]
                    if (2 * st + c) % 2 == 0:
                        nc.scalar.activation(
                            dst, ps.rearrange("p (h e) -> p h e", e=64),
                            AF.Copy, scale=cst[:, 16 + st:17 + st])
                    else:
                        nc.vector.tensor_scalar_mul(
                            dst, ps.rearrange("p (h e) -> p h e", e=64),
                            cst[:, 16 + st:17 + st])

        # ---- interleaved Q/K projections + attention, by head-group tile ----
        with (
            tc.tile_pool(name="ptsp", bufs=4) as ptsp,
            tc.tile_pool(name="rp", bufs=3) as rp,
            tc.tile_pool(name="rbp", bufs=3) as rbp,
            tc.tile_pool(name="lpsp", bufs=2, space="PSUM") as lpsp,
            tc.tile_pool(name="pvp", bufs=2, space="PSUM") as pvp,
        ):
            def emit_proj(tg):
                for kind in ("q", "k"):
                    w_sb = wq8 if kind == "q" else wk8
                    x_sb = xq8 if kind == "q" else xk8
                    dst = qt if kind == "q" else kt
                    bcol = 0 if kind == "q" else 8
                    for a in (2 * tg, 2 * tg + 1):
                        for sc in range(2):
                            ps = gp.tile([128, 512], F32, name=f"ps_{kind}_{a}_{sc}", tag="gp")
                            for t in range(4):
                                nc.tensor.matmul(
                                    ps, w_sb[:, t, :, a * 128:(a + 1) * 128],
                                    x_sb[:, t, :, sc * 512:(sc + 1) * 512],
                                    start=(t == 0), stop=(t == 3), perf_mode=DR)
                            nc.scalar.activation(
                                dst[a // 2][:, a % 2, sc * 512:(sc + 1) * 512], ps,
                                AF.Identity, bias=cst[:, bcol + a:bcol + a + 1])

            # deferred drains: (kind, payload) emitted mid-way through the NEXT
            # head's exp stream so their psum/broadcast inputs are certainly
            # ready when the in-order ACT/DVE queues reach them
            pending = []

            def flush_pending():
                for kind, args in pending:
                    if kind == "oa":
                        st, dc, psA = args
                        nc.scalar.activation(oa[st][:, dc * 512:(dc + 1) * 512],
                                             psA, AF.Copy, scale=CTX_S * WOB_S)
                    else:
                        h2, qc2, pv2, rbc2 = args
                        ct, cb = h2 // 4, (h2 // 2) % 2
                        nc.vector.tensor_mul(
                            ctxp[ct][64 * (h2 % 2):64 * (h2 % 2) + 64, cb,
                                     qc2 * 512:(qc2 + 1) * 512],
                            pv2[0:64, :], rbc2)
                pending.clear()

            for tg in range(4):
                emit_proj(tg)
                for h in range(4 * tg, 4 * tg + 4):
                    jj = h % 4
                    base = 32 * jj
                    pts = ptsp.tile([128, 4, 2, S], F8, name=f"pts{h}", tag="pts")
                    for qc in range(2):
                        for j in range(4):
                            lps = lpsp.tile([128, 2, 512], F32, name=f"lps_{h}_{qc}_{j}", tag="lps")
                            for kk in range(2):
                                kc = 2 * j + kk
                                nc.tensor.matmul(
                                    lps[:, kk, :],
                                    kt[tg][base:base + 32, :, kc * 128:(kc + 1) * 128],
                                    qt[tg][base:base + 32, :, qc * 512:(qc + 1) * 512],
                                    start=True, stop=True, perf_mode=DR,
                                    tile_position=(base, 0))
                            eng = exp_sched[exp_i[0] % 128]
                            exp_i[0] += 1
                            dst = pts[:, j, :, qc * 512:(qc + 1) * 512]
                            if eng == 0:
                                nc.scalar.activation(dst, lps, AF.Exp, scale=SCALE)
                            else:
                                nc.vector.tensor_scalar(dst.bitcast(I8), lps,
                                                        A_SCH * SCALE, B_SCH,
                                                        op0=ALU.mult, op1=ALU.add)
                            if qc == 0 and j == 1:
                                flush_pending()
                    if h == 0:
                        emit_projV()
                    for qc in range(2):
                        pv = pvp.tile([65, 512], F32, name=f"pv_{h}_{qc}", tag="pv")
                        for t in range(4):
                            nc.tensor.matmul(pv, v65[t][:, :, h, :],
                                             pts[:, t, :, qc * 512:(qc + 1) * 512],
                                             start=(t == 0), stop=(t == 3), perf_mode=DR)
                        recip = rp.tile([1, 512], F32, name=f"rc_{h}_{qc}", tag="rc")
                        nc.vector.reciprocal(recip, pv[64:65, :])
                        rbc = rbp.tile([64, 512], F32, name=f"rb_{h}_{qc}", tag="rb")
                        nc.gpsimd.partition_broadcast(rbc, recip, channels=64)
                        pending.append(("mul", (h, qc, pv, rbc)))
                    if dbg and h == 0:
                        nc.sync.dma_start(dpts_d[:, :, :, :], pts)
                    st, dc = (h) // 2, (h) % 2
                    psA = gp.tile([128, 512], F32, name=f"psA_{st}_{dc}", tag="gp")
                    for t8 in range(8):
                        nc.tensor.matmul(psA, xqb[:, t8, st * 128:(st + 1) * 128],
                                         wot[:, t8, dc * 512:(dc + 1) * 512],
                                         start=(t8 == 0), stop=False)
                    nc.tensor.matmul(psA, ones1, bor[:, dc * 512:(dc + 1) * 512],
                                     start=False, stop=True)
                    pending.append(("oa", (st, dc, psA)))
            flush_pending()

        psum_es.close()
        ap_es.close()
        if dbg:
            for t in range(4):
                nc.sync.dma_start(dqt_d[t], qt[t])
                nc.sync.dma_start(dkt_d[t], kt[t])
                nc.sync.dma_start(dv65_d[t], v65[t])
                nc.sync.dma_start(dctx_d[t], ctxp[t])
            for st in range(8):
                nc.sync.dma_start(doa_d[st], oa[st])

        # ============ Phase C: ctx @ Wo_bot + combine ============
        with (
            tc.tile_pool(name="wobp", bufs=1) as wobp,
            tc.tile_pool(name="outp", bufs=4) as outp,
            tc.tile_pool(name="php", bufs=4, space="PSUM") as php,
        ):
            wob8 = wobp.tile([128, 4, 2, D], F8, name="wob8")
            nc.sync.dma_start(wob8, wob8_d[:, :, :, :])
            for st in range(8):
                osb = outp.tile([128, D], F32, name=f"osb{st}", tag="osb")
                for dc in range(2):
                    ps2 = php.tile([128, 512], F32, name=f"ps2_{st}_{dc}", tag="php")
                    for j in range(4):
                        nc.tensor.matmul(ps2, ctxp[j][:, :, st * 128:(st + 1) * 128],
                                         wob8[:, j, :, dc * 512:(dc + 1) * 512],
                                         start=(j == 0), stop=False, perf_mode=DR)
                    nc.tensor.matmul(ps2, idm, oa[st][:, dc * 512:(dc + 1) * 512],
                                     start=False, stop=True)
                    nc.scalar.activation(osb[:, dc * 512:(dc + 1) * 512], ps2,
                                         AF.Copy, scale=OUT_S)
                nc.sync.dma_start(out_d[st * 128:(st + 1) * 128, :], osb)

        es.close()

    nc.finalize()
    return nc


_NC_CACHE = None


def _get_nc():
    global _NC_CACHE
    if _NC_CACHE is None:
        _NC_CACHE = build_nc()
    return _NC_CACHE


def _perm():
    pi = np.empty(D, np.int64)
    for a in range(8):
        for q in range(128):
            pi[a * 128 + q] = 64 * (4 * (a // 2) + q // 32) + 32 * (a % 2) + q % 32
    return pi


def _pair4(x):
    # [1024, N] -> [128, 4, 2, N] with row 128*(2t+b)+p at [p, t, b]
    n = x.shape[1]
    return np.ascontiguousarray(x.reshape(4, 2, 128, n).transpose(2, 0, 1, 3))


def kernel(**inputs):
    global LAST_EXEC_NS
    v = np.asarray(inputs["v"], np.float32)
    k = np.asarray(inputs["k"], np.float32)
    q_in = np.asarray(inputs["q_in"], np.float32)
    mask = np.asarray(inputs["mask"], np.float32)
    wq_w = np.asarray(inputs["wq_w"], np.float32)
    wq_b = np.asarray(inputs["wq_b"], np.float32)
    wk_w = np.asarray(inputs["wk_w"], np.float32)
    wk_b = np.asarray(inputs["wk_b"], np.float32)
    wv_w = np.asarray(inputs["wv_w"], np.float32)
    wv_b = np.asarray(inputs["wv_b"], np.float32)
    wo_w = np.asarray(inputs["wo_w"], np.float32)
    wo_b = np.asarray(inputs["wo_b"], np.float32)

    pi = _perm()
    wq8 = _pair4(wq_w[:, pi].astype(NP8))
    wk8 = _pair4(wk_w[:, pi].astype(NP8))
    wv8 = _pair4(wv_w.astype(NP8))
    wob8 = _pair4((wo_w[D:] * WOB_S).astype(NP8))
    wot = np.ascontiguousarray(
        wo_w[:D].reshape(8, 128, D).transpose(1, 0, 2)).astype(NPBF)
    bor = wo_b.reshape(1, D).astype(NPBF)
    bqp = wq_b[pi].reshape(8, 128).T          # [128, 8]
    bkp = wk_b[pi].reshape(8, 128).T

    in_maps = []
    for bi in range(B):
        m = np.exp(np.float32(-1e9) * mask[bi, 0, 0, :]).astype(np.float32)
        m_st = m.reshape(8, 128).T            # [128, 8]
        cst = np.concatenate([bqp, bkp, m_st], axis=1).astype(np.float32)
        xqT = np.ascontiguousarray(q_in[bi].T)
        in_maps.append({
            "xq8": _pair4(xqT.astype(NP8)),
            "xk8": _pair4(k[bi].T.astype(NP8)),
            "xv8": _pair4(v[bi].T.astype(NP8)),
            "wq8": wq8, "wk8": wk8, "wv8": wv8, "wob8": wob8,
            "xqb": np.ascontiguousarray(xqT.reshape(8, 128, S).transpose(1, 0, 2)).astype(NPBF),
            "wot": wot, "cst": np.ascontiguousarray(cst),
            "bvr": wv_b.reshape(1, D).astype(NPBF), "bor": bor,
            "idm": np.eye(128, dtype=np.float32).astype(NPBF),
        })

    nc = _get_nc()
    trace = os.environ.get("MHA_TRACE", "0") == "1"
    res = run_bass_kernel_spmd(nc, in_maps, core_ids=list(range(B)), trace=trace)
    LAST_EXEC_NS = res.exec_time_ns
    globals()["LAST_RES"] = res
    return np.stack([r["out"] for r in res.results], axis=0)


# revision 75
# speedup vs baseline: 1.0020x; 1.0020x over previous
"""Trainium2 Bass kernel for nn_MultiHeadAttention_73607149519012.

MHA: B=8, S=1024, D=1024, H=16 heads, depth=64, fp32 in/out.
Sharding: data-parallel over batch -- one batch element per NeuronCore (8 cores).

fp8e4 DoubleRow matmuls everywhere except the dominant q_in @ Wo_top half
(bf16).  DoubleRow packs two K-tiles per instruction at 0.5 cycles/row, so the
fp8 matmuls run ~4x faster than bf16 per MAC.  The attention path contributes
only ~2% of the output magnitude (ctx std ~0.02 vs q_in std ~1), so fp8 noise
there is diluted ~50x and the dominant half stays bf16.

Layouts (all host-prepared, no on-device transposes):
  Q/K proj : psum[dout',s] = Wq'^T x^T  with Wq' column-permuted so each psum
             chunk drains 1:1 into the DoubleRow logits layout
             qt[tile][32*(h%4)+d%32, d//32, s]  (4 heads per 128-partition tile)
  V proj   : psum[s,dv] -> v65p[kt//2][s, kt%2, head, 0:64]; column 64 holds
             mask[kpos]/32 so the PV matmul accumulates rowsum/32 in row 64
  logits   : DR lhsT=KT[32,2,128], rhs=QT[32,2,512] at tile_position (32*(h%4),0)
             -> psum[kpos128, q512];  exp has NO mask term (mask rides V rows)
  exp      : split ACT (exact Exp) / DVE (Schraudolph int8(a*x+b) bitcast fp8,
             rms ~3%, mean ~0; GPSIMD cannot read PSUM so Pool gets none)
  PV       : DR lhsT=v65p[128,2,65], rhs=pts[128,2,512] -> psum[65, 2, q512]
             (both q-chunks in one 2-bank tile); row 64 = rowsum/32;
             reciprocal -> Pool broadcast -> DVE mul gives ctx*32 in fp8
  out      : psum[s,dout] = x@Wo_top (bf16, + K=1 ones-row matmul adding bo)
             drained by ACT to bf16 oa; then psum2 = (32ctx)@(64Wo_bot) fp8 DR
             and final out = psum2/2048 + oa  (scalar_tensor_tensor)

Emission interleaves phases so the exp stream (the ACT/DVE bottleneck) starts
as early as possible: V proj first, then per-tile [Q chunks, K chunks, 4 heads].
"""

import os
from contextlib import ExitStack

import numpy as np
import ml_dtypes

import concourse.mybir as mybir
import concourse.tile as tile
from concourse import bacc
from concourse.bass_utils import run_bass_kernel_spmd

F32 = mybir.dt.float32
BF16 = mybir.dt.bfloat16
F8 = mybir.dt.float8e4
I8 = mybir.dt.int8
AF = mybir.ActivationFunctionType
ALU = mybir.AluOpType
DR = mybir.MatmulPerfMode.DoubleRow

NP8 = ml_dtypes.float8_e4m3
NPBF = ml_dtypes.bfloat16

D = 1024
S = 1024
H = 16
DEP = 64
B = 8
SCALE = 1.0 / 8.0          # 1/sqrt(DEP)
A_SCH = 8.0 / np.log(2.0)  # Schraudolph slope for 3-bit-mantissa fp8
B_SCH = 56.0 - 0.45        # bias 7*8, calibrated -0.45 to zero the mean error
CTX_S = 32.0               # ctx scaled by 32 into fp8 (ones column = 1/32)
WOB_S = 64.0               # Wo_bot scaled by 64 into fp8
OUT_S = 1.0 / (CTX_S * WOB_S)

# exp engine split ACT:DVE:(Pool unused), tunable for balance
_EW = os.environ.get("MHA_EXP_W", "80,48,0")
EXP_W = tuple(int(x) for x in _EW.split(","))

LAST_EXEC_NS = None
LAST_RES = None


def _exp_schedule():
    total = sum(EXP_W)
    counts = [w * 128 // total for w in EXP_W]
    while sum(counts) < 128:
        counts[int(np.argmax([w / (c + 1) for w, c in zip(EXP_W, counts)]))] += 1
    sched = []
    acc = [0.0, 0.0, 0.0]
    for _ in range(128):
        for i in range(3):
            acc[i] += counts[i] / 128.0
        pick = int(np.argmax(acc))
        acc[pick] -= 1.0
        sched.append(pick)
    return sched


def build_nc():
    nc = bacc.Bacc(None, target_bir_lowering=False)

    xq8_d = nc.dram_tensor("xq8", [128, 4, 2, S], F8, kind="ExternalInput")
    xk8_d = nc.dram_tensor("xk8", [128, 4, 2, S], F8, kind="ExternalInput")
    xv8_d = nc.dram_tensor("xv8", [128, 4, 2, S], F8, kind="ExternalInput")
    wq8_d = nc.dram_tensor("wq8", [128, 4, 2, D], F8, kind="ExternalInput")
    wk8_d = nc.dram_tensor("wk8", [128, 4, 2, D], F8, kind="ExternalInput")
    wv8_d = nc.dram_tensor("wv8", [128, 4, 2, D], F8, kind="ExternalInput")
    wob8_d = nc.dram_tensor("wob8", [128, 4, 2, D], F8, kind="ExternalInput")
    xqb_d = nc.dram_tensor("xqb", [128, 8, S], BF16, kind="ExternalInput")
    wot_d = nc.dram_tensor("wot", [128, 8, D], BF16, kind="ExternalInput")
    # cst cols: 0-7 bq' (permuted), 8-15 bk' (permuted), 16-23 mprime per st
    cst_d = nc.dram_tensor("cst", [128, 24], F32, kind="ExternalInput")
    bvr_d = nc.dram_tensor("bvr", [1, D], BF16, kind="ExternalInput")
    bor_d = nc.dram_tensor("bor", [1, D], BF16, kind="ExternalInput")
    idm_d = nc.dram_tensor("idm", [128, 128], BF16, kind="ExternalInput")
    out_d = nc.dram_tensor("out", [S, D], F32, kind="ExternalOutput")

    dbg = os.environ.get("MHA_DBG", "0") == "1"
    if dbg:
        dqt_d = nc.dram_tensor("dqt", [4, 128, 2, S], F8, kind="ExternalOutput")
        dkt_d = nc.dram_tensor("dkt", [4, 128, 2, S], F8, kind="ExternalOutput")
        dv65_d = nc.dram_tensor("dv65", [4, 128, 2, H, 65], F8, kind="ExternalOutput")
        dpts_d = nc.dram_tensor("dpts", [128, 4, 2, S], F8, kind="ExternalOutput")
        dctx_d = nc.dram_tensor("dctx", [4, 128, 2, S], F8, kind="ExternalOutput")
        doa_d = nc.dram_tensor("doa", [8, 128, D], BF16, kind="ExternalOutput")

    exp_sched = _exp_schedule()
    exp_i = [0]

    with tile.TileContext(nc) as tc:
        es = ExitStack()
        cp = es.enter_context(tc.tile_pool(name="cp", bufs=1))
        ap_es = ExitStack()
        ap = ap_es.enter_context(tc.tile_pool(name="ap", bufs=1))

        # ---- persistent tiles ----
        cst = cp.tile([128, 24], F32, name="cst")
        xqb = cp.tile([128, 8, S], BF16, name="xqb")
        wot = cp.tile([128, 8, D], BF16, name="wot")
        bvr = cp.tile([1, D], BF16, name="bvr")
        bor = cp.tile([1, D], BF16, name="bor")
        idm = cp.tile([128, 128], BF16, name="idm")
        ones1 = cp.tile([1, 128], BF16, name="ones1")
        qt = [cp.tile([128, 2, S], F8, name=f"qt{t}", tag=f"qt{t}") for t in range(4)]
        kt = [cp.tile([128, 2, S], F8, name=f"kt{t}", tag=f"kt{t}") for t in range(4)]
        # column 64 holds mask/32 so the PV matmul accumulates the softmax
        # denominator (rowsum/32) in psum row 64
        v65 = [cp.tile([128, 2, H, 65], F8, name=f"v65_{t}", tag=f"v65_{t}") for t in range(4)]
        ctxp = [cp.tile([128, 2, S], F8, name=f"ctx{t}", tag=f"ctx{t}") for t in range(4)]
        oa = [cp.tile([128, D], BF16, name=f"oa{t}", tag=f"oa{t}") for t in range(8)]

        xv8 = ap.tile([128, 4, 2, S], F8, name="xv8")
        wv8 = ap.tile([128, 4, 2, D], F8, name="wv8")
        xq8 = ap.tile([128, 4, 2, S], F8, name="xq8")
        wq8 = ap.tile([128, 4, 2, D], F8, name="wq8")
        xk8 = ap.tile([128, 4, 2, S], F8, name="xk8")
        wk8 = ap.tile([128, 4, 2, D], F8, name="wk8")

        # loads in need-order, big fp8 tensors split per k-tile-pair so the
        # first accumulation steps can start before the full tensor lands
        nc.sync.dma_start(cst, cst_d[:, :])
        nc.sync.dma_start(bvr, bvr_d[:, :])
        nc.gpsimd.memset(ones1, 1.0)
        for t in range(4):
            nc.sync.dma_start(xv8[:, t], xv8_d[:, t])
            nc.sync.dma_start(wv8[:, t], wv8_d[:, t])
        for t in range(4):
            nc.sync.dma_start(xq8[:, t], xq8_d[:, t])
            nc.sync.dma_start(wq8[:, t], wq8_d[:, t])
        for t in range(4):
            nc.sync.dma_start(xk8[:, t], xk8_d[:, t])
            nc.sync.dma_start(wk8[:, t], wk8_d[:, t])
        nc.sync.dma_start(xqb, xqb_d[:, :, :])
        nc.sync.dma_start(wot, wot_d[:, :, :])
        nc.sync.dma_start(bor, bor_d[:, :])
        nc.sync.dma_start(idm, idm_d[:, :])

        # ones/mask column of v65 (column 64 = mprime/32 per kpos), on Pool
        for t in range(4):
            nc.gpsimd.memset(v65[t][:, :, :, 64:65], 1.0 / CTX_S)
        for st in range(8):
            t, b = st // 2, st % 2
            nc.gpsimd.tensor_scalar_mul(
                v65[t][:, b, :, 64:65], v65[t][:, b, :, 64:65],
                cst[:, 16 + st:17 + st])

        psum_es = ExitStack()
        gp = psum_es.enter_context(tc.tile_pool(name="gp", bufs=2, space="PSUM"))

        def emit_outA(gi):
            st, dc = gi // 2, gi % 2
            psA = gp.tile([128, 512], F32, name=f"psA_{st}_{dc}", tag="gp")
            for t8 in range(8):
                nc.tensor.matmul(psA, xqb[:, t8, st * 128:(st + 1) * 128],
                                 wot[:, t8, dc * 512:(dc + 1) * 512],
                                 start=(t8 == 0), stop=False)
            nc.tensor.matmul(psA, ones1, bor[:, dc * 512:(dc + 1) * 512],
                             start=False, stop=True)
            # oa is stored pre-scaled by CTX_S*WOB_S so phase C can add it back
            # into the ph3 psum with a single identity-lhsT matmul
            nc.scalar.activation(oa[st][:, dc * 512:(dc + 1) * 512], psA, AF.Copy,
                                 scale=CTX_S * WOB_S)

        # ---- V projection first (own psum pool); drains on ACT, which is
        # otherwise idle during the load-bound startup. Bias rides a K=1
        # ones-row matmul, the mask a per-partition Copy scale.
        with tc.tile_pool(name="ppV", bufs=4, space="PSUM") as ppV:
            for st in range(8):
                t, b = st // 2, st % 2
                for c in range(2):
                    ps = ppV.tile([128, 512], F32, name=f"ps_v_{st}_{c}", tag="ppV")
                    for t4 in range(4):
                        nc.tensor.matmul(
                            ps, xv8[:, t4, :, st * 128:(st + 1) * 128],
                            wv8[:, t4, :, c * 512:(c + 1) * 512],
                            start=(t4 == 0), stop=False, perf_mode=DR)
                    nc.tensor.matmul(ps, ones1, bvr[:, c * 512:(c + 1) * 512],
                                     start=False, stop=True)
                    nc.scalar.activation(
                        v65[t][:, b, c * 8:(c + 1) * 8, 0:64],
                        ps.rearrange("p (h e) -> p h e", e=64),
                        AF.Copy, scale=cst[:, 16 + st:17 + st])

        # ---- interleaved Q/K projections + attention, by head-group tile ----
        with (
            tc.tile_pool(name="ptsp", bufs=4) as ptsp,
            tc.tile_pool(name="rp", bufs=3) as rp,
            tc.tile_pool(name="rbp", bufs=3) as rbp,
            tc.tile_pool(name="lpsp", bufs=2, space="PSUM") as lpsp,
            tc.tile_pool(name="pvp", bufs=2, space="PSUM") as pvp,
        ):
            def emit_proj(tg):
                for kind in ("q", "k"):
                    w_sb = wq8 if kind == "q" else wk8
                    x_sb = xq8 if kind == "q" else xk8
                    dst = qt if kind == "q" else kt
                    bcol = 0 if kind == "q" else 8
                    for a in (2 * tg, 2 * tg + 1):
                        for sc in range(2):
                            ps = gp.tile([128, 512], F32, name=f"ps_{kind}_{a}_{sc}", tag="gp")
                            for t in range(4):
                                nc.tensor.matmul(
                                    ps, w_sb[:, t, :, a * 128:(a + 1) * 128],
                                    x_sb[:, t, :, sc * 512:(sc + 1) * 512],
                                    start=(t == 0), stop=(t == 3), perf_mode=DR)
                            nc.scalar.activation(
                                dst[a // 2][:, a % 2, sc * 512:(sc + 1) * 512], ps,
                                AF.Identity, bias=cst[:, bcol + a:bcol + a + 1])

            # deferred drains: (kind, payload) emitted mid-way through the NEXT
            # head's exp stream so their psum/broadcast inputs are certainly
            # ready when the in-order ACT/DVE queues reach them
            pending = []

            def flush_pending():
                for kind, args in pending:
                    if kind == "oa":
                        st, dc, psA = args
                        nc.scalar.activation(oa[st][:, dc * 512:(dc + 1) * 512],
                                             psA, AF.Copy, scale=CTX_S * WOB_S)
                    else:
                        h2, qc2, pv2, rbc2 = args
                        ct, cb = h2 // 4, (h2 // 2) % 2
                        nc.vector.tensor_mul(
                            ctxp[ct][64 * (h2 % 2):64 * (h2 % 2) + 64, cb,
                                     qc2 * 512:(qc2 + 1) * 512],
                            pv2[0:64, :], rbc2)
                pending.clear()

            for tg in range(4):
                emit_proj(tg)
                for h in range(4 * tg, 4 * tg + 4):
                    jj = h % 4
                    base = 32 * jj
                    pts = ptsp.tile([128, 4, 2, S], F8, name=f"pts{h}", tag="pts")
                    for qc in range(2):
                        for j in range(4):
                            lps = lpsp.tile([128, 2, 512], F32, name=f"lps_{h}_{qc}_{j}", tag="lps")
                            for kk in range(2):
                                kc = 2 * j + kk
                                nc.tensor.matmul(
                                    lps[:, kk, :],
                                    kt[tg][base:base + 32, :, kc * 128:(kc + 1) * 128],
                                    qt[tg][base:base + 32, :, qc * 512:(qc + 1) * 512],
                                    start=True, stop=True, perf_mode=DR,
                                    tile_position=(base, 0))
                            eng = exp_sched[exp_i[0] % 128]
                            exp_i[0] += 1
                            dst = pts[:, j, :, qc * 512:(qc + 1) * 512]
                            if eng == 0:
                                nc.scalar.activation(dst, lps, AF.Exp, scale=SCALE)
                            else:
                                nc.vector.tensor_scalar(dst.bitcast(I8), lps,
                                                        A_SCH * SCALE, B_SCH,
                                                        op0=ALU.mult, op1=ALU.add)
                            if qc == 0 and j == 1:
                                flush_pending()
                    for qc in range(2):
                        pv = pvp.tile([65, 512], F32, name=f"pv_{h}_{qc}", tag="pv")
                        for t in range(4):
                            nc.tensor.matmul(pv, v65[t][:, :, h, :],
                                             pts[:, t, :, qc * 512:(qc + 1) * 512],
                                             start=(t == 0), stop=(t == 3), perf_mode=DR)
                        recip = rp.tile([1, 512], F32, name=f"rc_{h}_{qc}", tag="rc")
                        nc.vector.reciprocal(recip, pv[64:65, :])
                        rbc = rbp.tile([64, 512], F32, name=f"rb_{h}_{qc}", tag="rb")
                        nc.gpsimd.partition_broadcast(rbc, recip, channels=64)
                        pending.append(("mul", (h, qc, pv, rbc)))
                    if dbg and h == 0:
                        nc.sync.dma_start(dpts_d[:, :, :, :], pts)
                    st, dc = (h) // 2, (h) % 2
                    psA = gp.tile([128, 512], F32, name=f"psA_{st}_{dc}", tag="gp")
                    for t8 in range(8):
                        nc.tensor.matmul(psA, xqb[:, t8, st * 128:(st + 1) * 128],
                                         wot[:, t8, dc * 512:(dc + 1) * 512],
                                         start=(t8 == 0), stop=False)
                    nc.tensor.matmul(psA, ones1, bor[:, dc * 512:(dc + 1) * 512],
                                     start=False, stop=True)
                    pending.append(("oa", (st, dc, psA)))
            flush_pending()

        psum_es.close()
        ap_es.close()
        if dbg:
            for t in range(4):
                nc.sync.dma_start(dqt_d[t], qt[t])
                nc.sync.dma_start(dkt_d[t], kt[t])
                nc.sync.dma_start(dv65_d[t], v65[t])
                nc.sync.dma_start(dctx_d[t], ctxp[t])
            for st in range(8):
                nc.sync.dma_start(doa_d[st], oa[st])

        # ============ Phase C: ctx @ Wo_bot + combine ============
        with (
            tc.tile_pool(name="wobp", bufs=1) as wobp,
            tc.tile_pool(name="outp", bufs=4) as outp,
            tc.tile_pool(name="php", bufs=4, space="PSUM") as php,
        ):
            wob8 = wobp.tile([128, 4, 2, D], F8, name="wob8")
            nc.sync.dma_start(wob8, wob8_d[:, :, :, :])
            for st in range(8):
                osb = outp.tile([128, D], F32, name=f"osb{st}", tag="osb")
                for dc in range(2):
                    ps2 = php.tile([128, 512], F32, name=f"ps2_{st}_{dc}", tag="php")
                    for j in range(4):
                        nc.tensor.matmul(ps2, ctxp[j][:, :, st * 128:(st + 1) * 128],
                                         wob8[:, j, :, dc * 512:(dc + 1) * 512],
                                         start=(j == 0), stop=False, perf_mode=DR)
                    nc.tensor.matmul(ps2, idm, oa[st][:, dc * 512:(dc + 1) * 512],
                                     start=False, stop=True)
                    nc.scalar.activation(osb[:, dc * 512:(dc + 1) * 512], ps2,
                                         AF.Copy, scale=OUT_S)
                nc.sync.dma_start(out_d[st * 128:(st + 1) * 128, :], osb)

        es.close()

    nc.finalize()
    return nc


_NC_CACHE = None


def _get_nc():
    global _NC_CACHE
    if _NC_CACHE is None:
        _NC_CACHE = build_nc()
    return _NC_CACHE


def _perm():
    pi = np.empty(D, np.int64)
    for a in range(8):
        for q in range(128):
            pi[a * 128 + q] = 64 * (4 * (a // 2) + q // 32) + 32 * (a % 2) + q % 32
    return pi


def _pair4(x):
    # [1024, N] -> [128, 4, 2, N] with row 128*(2t+b)+p at [p, t, b]
    n = x.shape[1]
    return np.ascontiguousarray(x.reshape(4, 2, 128, n).transpose(2, 0, 1, 3))


def kernel(**inputs):
    global LAST_EXEC_NS
    v = np.asarray(inputs["v"], np.float32)
    k = np.asarray(inputs["k"], np.float32)
    q_in = np.asarray(inputs["q_in"], np.float32)
    mask = np.asarray(inputs["mask"], np.float32)
    wq_w = np.asarray(inputs["wq_w"], np.float32)
    wq_b = np.asarray(inputs["wq_b"], np.float32)
    wk_w = np.asarray(inputs["wk_w"], np.float32)
    wk_b = np.asarray(inputs["wk_b"], np.float32)
    wv_w = np.asarray(inputs["wv_w"], np.float32)
    wv_b = np.asarray(inputs["wv_b"], np.float32)
    wo_w = np.asarray(inputs["wo_w"], np.float32)
    wo_b = np.asarray(inputs["wo_b"], np.float32)

    pi = _perm()
    wq8 = _pair4(wq_w[:, pi].astype(NP8))
    wk8 = _pair4(wk_w[:, pi].astype(NP8))
    wv8 = _pair4(wv_w.astype(NP8))
    wob8 = _pair4((wo_w[D:] * WOB_S).astype(NP8))
    wot = np.ascontiguousarray(
        wo_w[:D].reshape(8, 128, D).transpose(1, 0, 2)).astype(NPBF)
    bor = wo_b.reshape(1, D).astype(NPBF)
    bqp = wq_b[pi].reshape(8, 128).T          # [128, 8]
    bkp = wk_b[pi].reshape(8, 128).T

    in_maps = []
    for bi in range(B):
        m = np.exp(np.float32(-1e9) * mask[bi, 0, 0, :]).astype(np.float32)
        m_st = m.reshape(8, 128).T            # [128, 8]
        cst = np.concatenate([bqp, bkp, m_st], axis=1).astype(np.float32)
        xqT = np.ascontiguousarray(q_in[bi].T)
        in_maps.append({
            "xq8": _pair4(xqT.astype(NP8)),
            "xk8": _pair4(k[bi].T.astype(NP8)),
            "xv8": _pair4(v[bi].T.astype(NP8)),
            "wq8": wq8, "wk8": wk8, "wv8": wv8, "wob8": wob8,
            "xqb": np.ascontiguousarray(xqT.reshape(8, 128, S).transpose(1, 0, 2)).astype(NPBF),
            "wot": wot, "cst": np.ascontiguousarray(cst),
            "bvr": wv_b.reshape(1, D).astype(NPBF), "bor": bor,
            "idm": np.eye(128, dtype=np.float32).astype(NPBF),
        })

    nc = _get_nc()
    trace = os.environ.get("MHA_TRACE", "0") == "1"
    res = run_bass_kernel_spmd(nc, in_maps, core_ids=list(range(B)), trace=trace)
    LAST_EXEC_NS = res.exec_time_ns
    globals()["LAST_RES"] = res
    return np.stack([r["out"] for r in res.results], axis=0)


# revision 77
# speedup vs baseline: 1.0101x; 1.0081x over previous
"""Trainium2 Bass kernel for nn_MultiHeadAttention_73607149519012.

MHA: B=8, S=1024, D=1024, H=16 heads, depth=64, fp32 in/out.
Sharding: data-parallel over batch -- one batch element per NeuronCore (8 cores).

fp8e4 DoubleRow matmuls everywhere except the dominant q_in @ Wo_top half
(bf16).  DoubleRow packs two K-tiles per instruction at 0.5 cycles/row, so the
fp8 matmuls run ~4x faster than bf16 per MAC.  The attention path contributes
only ~2% of the output magnitude (ctx std ~0.02 vs q_in std ~1), so fp8 noise
there is diluted ~50x and the dominant half stays bf16.

Layouts (all host-prepared, no on-device transposes):
  Q/K proj : psum[dout',s] = Wq'^T x^T  with Wq' column-permuted so each psum
             chunk drains 1:1 into the DoubleRow logits layout
             qt[tile][32*(h%4)+d%32, d//32, s]  (4 heads per 128-partition tile)
  V proj   : psum[s,dv] -> v65p[kt//2][s, kt%2, head, 0:64]; column 64 holds
             mask[kpos]/32 so the PV matmul accumulates rowsum/32 in row 64
  logits   : DR lhsT=KT[32,2,128], rhs=QT[32,2,512] at tile_position (32*(h%4),0)
             -> psum[kpos128, q512];  exp has NO mask term (mask rides V rows)
  exp      : split ACT (exact Exp) / DVE (Schraudolph int8(a*x+b) bitcast fp8,
             rms ~3%, mean ~0; GPSIMD cannot read PSUM so Pool gets none)
  PV       : DR lhsT=v65p[128,2,65], rhs=pts[128,2,512] -> psum[65, 2, q512]
             (both q-chunks in one 2-bank tile); row 64 = rowsum/32;
             reciprocal -> Pool broadcast -> DVE mul gives ctx*32 in fp8
  out      : psum[s,dout] = x@Wo_top (bf16, + K=1 ones-row matmul adding bo)
             drained by ACT to bf16 oa; then psum2 = (32ctx)@(64Wo_bot) fp8 DR
             and final out = psum2/2048 + oa  (scalar_tensor_tensor)

Emission interleaves phases so the exp stream (the ACT/DVE bottleneck) starts
as early as possible: V proj first, then per-tile [Q chunks, K chunks, 4 heads].
"""

import os
from contextlib import ExitStack

import numpy as np
import ml_dtypes

import concourse.mybir as mybir
import concourse.tile as tile
from concourse import bacc
from concourse.bass_utils import run_bass_kernel_spmd

F32 = mybir.dt.float32
BF16 = mybir.dt.bfloat16
F8 = mybir.dt.float8e4
I8 = mybir.dt.int8
AF = mybir.ActivationFunctionType
ALU = mybir.AluOpType
DR = mybir.MatmulPerfMode.DoubleRow

NP8 = ml_dtypes.float8_e4m3
NPBF = ml_dtypes.bfloat16

D = 1024
S = 1024
H = 16
DEP = 64
B = 8
SCALE = 1.0 / 8.0          # 1/sqrt(DEP)
A_SCH = 8.0 / np.log(2.0)  # Schraudolph slope for 3-bit-mantissa fp8
B_SCH = 56.0 - 0.45        # bias 7*8, calibrated -0.45 to zero the mean error
CTX_S = 32.0               # ctx scaled by 32 into fp8 (ones column = 1/32)
WOB_S = 64.0               # Wo_bot scaled by 64 into fp8
OUT_S = 1.0 / (CTX_S * WOB_S)

# exp engine split ACT:DVE:(Pool unused), tunable for balance
_EW = os.environ.get("MHA_EXP_W", "72,56,0")
EXP_W = tuple(int(x) for x in _EW.split(","))

LAST_EXEC_NS = None
LAST_RES = None


def _exp_schedule():
    total = sum(EXP_W)
    counts = [w * 128 // total for w in EXP_W]
    while sum(counts) < 128:
        counts[int(np.argmax([w / (c + 1) for w, c in zip(EXP_W, counts)]))] += 1
    sched = []
    acc = [0.0, 0.0, 0.0]
    for _ in range(128):
        for i in range(3):
            acc[i] += counts[i] / 128.0
        pick = int(np.argmax(acc))
        acc[pick] -= 1.0
        sched.append(pick)
    return sched


def build_nc():
    nc = bacc.Bacc(None, target_bir_lowering=False)

    xq8_d = nc.dram_tensor("xq8", [128, 4, 2, S], F8, kind="ExternalInput")
    xk8_d = nc.dram_tensor("xk8", [128, 4, 2, S], F8, kind="ExternalInput")
    xv8_d = nc.dram_tensor("xv8", [128, 4, 2, S], F8, kind="ExternalInput")
    wq8_d = nc.dram_tensor("wq8", [128, 4, 2, D], F8, kind="ExternalInput")
    wk8_d = nc.dram_tensor("wk8", [128, 4, 2, D], F8, kind="ExternalInput")
    wv8_d = nc.dram_tensor("wv8", [128, 4, 2, D], F8, kind="ExternalInput")
    wob8_d = nc.dram_tensor("wob8", [128, 4, 2, D], F8, kind="ExternalInput")
    xqb_d = nc.dram_tensor("xqb", [128, 8, S], BF16, kind="ExternalInput")
    wot_d = nc.dram_tensor("wot", [128, 8, D], BF16, kind="ExternalInput")
    # cst cols: 0-7 bq' (permuted), 8-15 bk' (permuted), 16-23 mprime per st
    cst_d = nc.dram_tensor("cst", [128, 24], F32, kind="ExternalInput")
    bvr_d = nc.dram_tensor("bvr", [1, D], BF16, kind="ExternalInput")
    bor_d = nc.dram_tensor("bor", [1, D], BF16, kind="ExternalInput")
    idm_d = nc.dram_tensor("idm", [128, 128], BF16, kind="ExternalInput")
    out_d = nc.dram_tensor("out", [S, D], F32, kind="ExternalOutput")

    dbg = os.environ.get("MHA_DBG", "0") == "1"
    if dbg:
        dqt_d = nc.dram_tensor("dqt", [4, 128, 2, S], F8, kind="ExternalOutput")
        dkt_d = nc.dram_tensor("dkt", [4, 128, 2, S], F8, kind="ExternalOutput")
        dv65_d = nc.dram_tensor("dv65", [4, 128, 2, H, 65], F8, kind="ExternalOutput")
        dpts_d = nc.dram_tensor("dpts", [128, 4, 2, S], F8, kind="ExternalOutput")
        dctx_d = nc.dram_tensor("dctx", [4, 128, 2, S], F8, kind="ExternalOutput")
        doa_d = nc.dram_tensor("doa", [8, 128, D], BF16, kind="ExternalOutput")

    exp_sched = _exp_schedule()
    exp_i = [0]

    with tile.TileContext(nc) as tc:
        es = ExitStack()
        cp = es.enter_context(tc.tile_pool(name="cp", bufs=1))
        ap_es = ExitStack()
        ap = ap_es.enter_context(tc.tile_pool(name="ap", bufs=1))

        # ---- persistent tiles ----
        cst = cp.tile([128, 24], F32, name="cst")
        xqb = cp.tile([128, 8, S], BF16, name="xqb")
        wot = cp.tile([128, 8, D], BF16, name="wot")
        bvr = cp.tile([1, D], BF16, name="bvr")
        bor = cp.tile([1, D], BF16, name="bor")
        idm = cp.tile([128, 128], BF16, name="idm")
        ones1 = cp.tile([1, 128], BF16, name="ones1")
        qt = [cp.tile([128, 2, S], F8, name=f"qt{t}", tag=f"qt{t}") for t in range(4)]
        kt = [cp.tile([128, 2, S], F8, name=f"kt{t}", tag=f"kt{t}") for t in range(4)]
        # column 64 holds mask/32 so the PV matmul accumulates the softmax
        # denominator (rowsum/32) in psum row 64
        v65 = [cp.tile([128, 2, H, 65], F8, name=f"v65_{t}", tag=f"v65_{t}") for t in range(4)]
        ctxp = [cp.tile([128, 2, S], F8, name=f"ctx{t}", tag=f"ctx{t}") for t in range(4)]
        oa = [cp.tile([128, D], BF16, name=f"oa{t}", tag=f"oa{t}") for t in range(8)]

        xv8 = ap.tile([128, 4, 2, S], F8, name="xv8")
        wv8 = ap.tile([128, 4, 2, D], F8, name="wv8")
        xq8 = ap.tile([128, 4, 2, S], F8, name="xq8")
        wq8 = ap.tile([128, 4, 2, D], F8, name="wq8")
        xk8 = ap.tile([128, 4, 2, S], F8, name="xk8")
        wk8 = ap.tile([128, 4, 2, D], F8, name="wk8")

        # loads in need-order, big fp8 tensors split per k-tile-pair so the
        # first accumulation steps can start before the full tensor lands
        nc.sync.dma_start(cst, cst_d[:, :])
        nc.sync.dma_start(bvr, bvr_d[:, :])
        nc.gpsimd.memset(ones1, 1.0)
        for t in range(4):
            nc.sync.dma_start(xv8[:, t], xv8_d[:, t])
            nc.sync.dma_start(wv8[:, t], wv8_d[:, t])
        for t in range(4):
            nc.sync.dma_start(xq8[:, t], xq8_d[:, t])
            nc.sync.dma_start(wq8[:, t], wq8_d[:, t])
        for t in range(4):
            nc.sync.dma_start(xk8[:, t], xk8_d[:, t])
            nc.sync.dma_start(wk8[:, t], wk8_d[:, t])
        nc.sync.dma_start(xqb, xqb_d[:, :, :])
        nc.sync.dma_start(wot, wot_d[:, :, :])
        nc.sync.dma_start(bor, bor_d[:, :])
        nc.sync.dma_start(idm, idm_d[:, :])

        # ones/mask column of v65 (column 64 = mprime/32 per kpos), on Pool
        for t in range(4):
            nc.gpsimd.memset(v65[t][:, :, :, 64:65], 1.0 / CTX_S)
        for st in range(8):
            t, b = st // 2, st % 2
            nc.gpsimd.tensor_scalar_mul(
                v65[t][:, b, :, 64:65], v65[t][:, b, :, 64:65],
                cst[:, 16 + st:17 + st])

        psum_es = ExitStack()
        gp = psum_es.enter_context(tc.tile_pool(name="gp", bufs=2, space="PSUM"))

        # ---- V projection first (own psum pool); drains on ACT, which is
        # otherwise idle during the load-bound startup. Bias rides a K=1
        # ones-row matmul, the mask a per-partition Copy scale.
        with tc.tile_pool(name="ppV", bufs=4, space="PSUM") as ppV:
            for st in range(8):
                t, b = st // 2, st % 2
                for c in range(2):
                    ps = ppV.tile([128, 512], F32, name=f"ps_v_{st}_{c}", tag="ppV")
                    for t4 in range(4):
                        nc.tensor.matmul(
                            ps, xv8[:, t4, :, st * 128:(st + 1) * 128],
                            wv8[:, t4, :, c * 512:(c + 1) * 512],
                            start=(t4 == 0), stop=False, perf_mode=DR)
                    nc.tensor.matmul(ps, ones1, bvr[:, c * 512:(c + 1) * 512],
                                     start=False, stop=True)
                    nc.scalar.activation(
                        v65[t][:, b, c * 8:(c + 1) * 8, 0:64],
                        ps.rearrange("p (h e) -> p h e", e=64),
                        AF.Copy, scale=cst[:, 16 + st:17 + st])

        # ---- interleaved Q/K projections + attention, by head-group tile ----
        with (
            tc.tile_pool(name="ptsp", bufs=4) as ptsp,
            tc.tile_pool(name="rp", bufs=3) as rp,
            tc.tile_pool(name="rbp", bufs=3) as rbp,
            tc.tile_pool(name="lpsp", bufs=2, space="PSUM") as lpsp,
            tc.tile_pool(name="pvp", bufs=2, space="PSUM") as pvp,
        ):
            def emit_proj(tg):
                for kind in ("q", "k"):
                    w_sb = wq8 if kind == "q" else wk8
                    x_sb = xq8 if kind == "q" else xk8
                    dst = qt if kind == "q" else kt
                    bcol = 0 if kind == "q" else 8
                    for a in (2 * tg, 2 * tg + 1):
                        for sc in range(2):
                            ps = gp.tile([128, 512], F32, name=f"ps_{kind}_{a}_{sc}", tag="gp")
                            for t in range(4):
                                nc.tensor.matmul(
                                    ps, w_sb[:, t, :, a * 128:(a + 1) * 128],
                                    x_sb[:, t, :, sc * 512:(sc + 1) * 512],
                                    start=(t == 0), stop=(t == 3), perf_mode=DR)
                            nc.scalar.activation(
                                dst[a // 2][:, a % 2, sc * 512:(sc + 1) * 512], ps,
                                AF.Identity, bias=cst[:, bcol + a:bcol + a + 1])

            # deferred drains: (kind, payload) emitted mid-way through the NEXT
            # head's exp stream so their psum/broadcast inputs are certainly
            # ready when the in-order ACT/DVE queues reach them
            pending = []

            def flush_pending():
                for kind, args in pending:
                    if kind == "oa":
                        st, dc, psA = args
                        nc.scalar.activation(oa[st][:, dc * 512:(dc + 1) * 512],
                                             psA, AF.Copy, scale=CTX_S * WOB_S)
                    else:
                        h2, qc2, pv2, rbc2 = args
                        ct, cb = h2 // 4, (h2 // 2) % 2
                        nc.vector.tensor_mul(
                            ctxp[ct][64 * (h2 % 2):64 * (h2 % 2) + 64, cb,
                                     qc2 * 512:(qc2 + 1) * 512],
                            pv2[0:64, :], rbc2)
                pending.clear()

            for tg in range(4):
                emit_proj(tg)
                for h in range(4 * tg, 4 * tg + 4):
                    jj = h % 4
                    base = 32 * jj
                    pts = ptsp.tile([128, 4, 2, S], F8, name=f"pts{h}", tag="pts")
                    for qc in range(2):
                        for j in range(4):
                            lps = lpsp.tile([128, 2, 512], F32, name=f"lps_{h}_{qc}_{j}", tag="lps")
                            for kk in range(2):
                                kc = 2 * j + kk
                                nc.tensor.matmul(
                                    lps[:, kk, :],
                                    kt[tg][base:base + 32, :, kc * 128:(kc + 1) * 128],
                                    qt[tg][base:base + 32, :, qc * 512:(qc + 1) * 512],
                                    start=True, stop=True, perf_mode=DR,
                                    tile_position=(base, 0))
                            eng = exp_sched[exp_i[0] % 128]
                            exp_i[0] += 1
                            dst = pts[:, j, :, qc * 512:(qc + 1) * 512]
                            if eng == 0:
                                nc.scalar.activation(dst, lps, AF.Exp, scale=SCALE)
                            else:
                                nc.vector.tensor_scalar(dst.bitcast(I8), lps,
                                                        A_SCH * SCALE, B_SCH,
                                                        op0=ALU.mult, op1=ALU.add)
                            if qc == 0 and j == 1:
                                flush_pending()
                    for qc in range(2):
                        pv = pvp.tile([65, 512], F32, name=f"pv_{h}_{qc}", tag="pv")
                        for t in range(4):
                            nc.tensor.matmul(pv, v65[t][:, :, h, :],
                                             pts[:, t, :, qc * 512:(qc + 1) * 512],
                                             start=(t == 0), stop=(t == 3), perf_mode=DR)
                        recip = rp.tile([1, 512], F32, name=f"rc_{h}_{qc}", tag="rc")
                        nc.vector.reciprocal(recip, pv[64:65, :])
                        rbc = rbp.tile([64, 512], F32, name=f"rb_{h}_{qc}", tag="rb")
                        nc.gpsimd.partition_broadcast(rbc, recip, channels=64)
                        pending.append(("mul", (h, qc, pv, rbc)))
                    if dbg and h == 0:
                        nc.sync.dma_start(dpts_d[:, :, :, :], pts)
                    st, dc = (h) // 2, (h) % 2
                    psA = gp.tile([128, 512], F32, name=f"psA_{st}_{dc}", tag="gp")
                    for t8 in range(8):
                        nc.tensor.matmul(psA, xqb[:, t8, st * 128:(st + 1) * 128],
                                         wot[:, t8, dc * 512:(dc + 1) * 512],
                                         start=(t8 == 0), stop=False)
                    nc.tensor.matmul(psA, ones1, bor[:, dc * 512:(dc + 1) * 512],
                                     start=False, stop=True)
                    pending.append(("oa", (st, dc, psA)))
            flush_pending()

        psum_es.close()
        ap_es.close()
        if dbg:
            for t in range(4):
                nc.sync.dma_start(dqt_d[t], qt[t])
                nc.sync.dma_start(dkt_d[t], kt[t])
                nc.sync.dma_start(dv65_d[t], v65[t])
                nc.sync.dma_start(dctx_d[t], ctxp[t])
            for st in range(8):
                nc.sync.dma_start(doa_d[st], oa[st])

        # ============ Phase C: ctx @ Wo_bot + combine ============
        with (
            tc.tile_pool(name="wobp", bufs=1) as wobp,
            tc.tile_pool(name="outp", bufs=4) as outp,
            tc.tile_pool(name="php", bufs=4, space="PSUM") as php,
        ):
            wob8 = wobp.tile([128, 4, 2, D], F8, name="wob8")
            nc.sync.dma_start(wob8, wob8_d[:, :, :, :])
            for st in range(8):
                osb = outp.tile([128, D], F32, name=f"osb{st}", tag="osb")
                for dc in range(2):
                    ps2 = php.tile([128, 512], F32, name=f"ps2_{st}_{dc}", tag="php")
                    for j in range(4):
                        nc.tensor.matmul(ps2, ctxp[j][:, :, st * 128:(st + 1) * 128],
                                         wob8[:, j, :, dc * 512:(dc + 1) * 512],
                                         start=(j == 0), stop=False, perf_mode=DR)
                    nc.tensor.matmul(ps2, idm, oa[st][:, dc * 512:(dc + 1) * 512],
                                     start=False, stop=True)
                    nc.scalar.activation(osb[:, dc * 512:(dc + 1) * 512], ps2,
                                         AF.Copy, scale=OUT_S)
                nc.sync.dma_start(out_d[st * 128:(st + 1) * 128, :], osb)

        es.close()

    nc.finalize()
    return nc


_NC_CACHE = None


def _get_nc():
    global _NC_CACHE
    if _NC_CACHE is None:
        _NC_CACHE = build_nc()
    return _NC_CACHE


def _perm():
    pi = np.empty(D, np.int64)
    for a in range(8):
        for q in range(128):
            pi[a * 128 + q] = 64 * (4 * (a // 2) + q // 32) + 32 * (a % 2) + q % 32
    return pi


def _pair4(x):
    # [1024, N] -> [128, 4, 2, N] with row 128*(2t+b)+p at [p, t, b]
    n = x.shape[1]
    return np.ascontiguousarray(x.reshape(4, 2, 128, n).transpose(2, 0, 1, 3))


def kernel(**inputs):
    global LAST_EXEC_NS
    v = np.asarray(inputs["v"], np.float32)
    k = np.asarray(inputs["k"], np.float32)
    q_in = np.asarray(inputs["q_in"], np.float32)
    mask = np.asarray(inputs["mask"], np.float32)
    wq_w = np.asarray(inputs["wq_w"], np.float32)
    wq_b = np.asarray(inputs["wq_b"], np.float32)
    wk_w = np.asarray(inputs["wk_w"], np.float32)
    wk_b = np.asarray(inputs["wk_b"], np.float32)
    wv_w = np.asarray(inputs["wv_w"], np.float32)
    wv_b = np.asarray(inputs["wv_b"], np.float32)
    wo_w = np.asarray(inputs["wo_w"], np.float32)
    wo_b = np.asarray(inputs["wo_b"], np.float32)

    pi = _perm()
    wq8 = _pair4(wq_w[:, pi].astype(NP8))
    wk8 = _pair4(wk_w[:, pi].astype(NP8))
    wv8 = _pair4(wv_w.astype(NP8))
    wob8 = _pair4((wo_w[D:] * WOB_S).astype(NP8))
    wot = np.ascontiguousarray(
        wo_w[:D].reshape(8, 128, D).transpose(1, 0, 2)).astype(NPBF)
    bor = wo_b.reshape(1, D).astype(NPBF)
    bqp = wq_b[pi].reshape(8, 128).T          # [128, 8]
    bkp = wk_b[pi].reshape(8, 128).T

    in_maps = []
    for bi in range(B):
        m = np.exp(np.float32(-1e9) * mask[bi, 0, 0, :]).astype(np.float32)
        m_st = m.reshape(8, 128).T            # [128, 8]
        cst = np.concatenate([bqp, bkp, m_st], axis=1).astype(np.float32)
        xqT = np.ascontiguousarray(q_in[bi].T)
        in_maps.append({
            "xq8": _pair4(xqT.astype(NP8)),
            "xk8": _pair4(k[bi].T.astype(NP8)),
            "xv8": _pair4(v[bi].T.astype(NP8)),
            "wq8": wq8, "wk8": wk8, "wv8": wv8, "wob8": wob8,
            "xqb": np.ascontiguousarray(xqT.reshape(8, 128, S).transpose(1, 0, 2)).astype(NPBF),
            "wot": wot, "cst": np.ascontiguousarray(cst),
            "bvr": wv_b.reshape(1, D).astype(NPBF), "bor": bor,
            "idm": np.eye(128, dtype=np.float32).astype(NPBF),
        })

    nc = _get_nc()
    trace = os.environ.get("MHA_TRACE", "0") == "1"
    res = run_bass_kernel_spmd(nc, in_maps, core_ids=list(range(B)), trace=trace)
    LAST_EXEC_NS = res.exec_time_ns
    globals()["LAST_RES"] = res
    return np.stack([r["out"] for r in res.results], axis=0)
